# revision 1
# baseline (speedup 1.0000x reference)
"""Multi-head causal attention (B=4, S=2048, D=1024, H=16) on 8 TRN2 NeuronCores.

Sharding: 2 cores per batch element, 8 heads (512 dims) per core.
Each core computes QKV projections for its head slice, causal attention,
and a partial output projection (its 512 rows of Wo). The host sums the
two partial outputs per batch element (the tensor-parallel all-reduce,
folded into the gather step).

Compute dtype: bf16 matmul inputs with fp32 PSUM accumulation (weights
and activations converted to bf16 host-side / on write-back).

Per-core dataflow (layouts chosen so no activation needs a transpose
after the initial X^T build, which itself is a DMA transpose):
  1. X^T [d x seq] chunks via DMA transpose (bf16).
  2. Q^T, K^T [dim(512) x seq] = W^T @ X^T, V [seq x dim] = X @ Wv,
     V stored bf16 with a ones column appended (the ones column makes
     the P@V matmul also emit softmax row sums).
  3. Per head: S^T tiles [ks,qs] = K @ Q^T, exp on ACT -> bf16 P^T,
     causal mask multiply on the 4 diagonal tiles, ctx^T accumulated as
     V_aug^T @ P^T (no P transpose needed in this orientation).
     Normalize with reciprocal of the sums row, broadcast across
     partitions via a ones-vector matmul.  bv added post-normalization
     (softmax rows sum to 1, so folding bv there is exact).
  4. out_partial = ctx^T.T @ Wo (+ bo on even cores only).
"""

import sys

import numpy as np


def _ensure_concourse():
    try:
        import concourse  # noqa: F401
    except ImportError:
        sys.path.insert(0, "/opt/trn_rl_repo")


_ensure_concourse()

B, S, D, H, HD = 4, 2048, 1024, 16, 64
DC = 512  # dims (= 8 heads) per core
N_CORES = 8

_nc_cache = None


def _build_bass():
    from contextlib import ExitStack

    import concourse.mybir as mybir
    import concourse.tile as tile
    from concourse import bacc

    f32 = mybir.dt.float32
    bf16 = mybir.dt.bfloat16
    Exp = mybir.ActivationFunctionType.Exp

    nc = bacc.Bacc(None, target_bir_lowering=False)

    x = nc.dram_tensor("x", [S, D], bf16, kind="ExternalInput")
    wq = nc.dram_tensor("wq", [D, DC], bf16, kind="ExternalInput")
    wk = nc.dram_tensor("wk", [D, DC], bf16, kind="ExternalInput")
    wv = nc.dram_tensor("wv", [D, DC], bf16, kind="ExternalInput")
    wo = nc.dram_tensor("wo", [DC, D], bf16, kind="ExternalInput")
    bq_d = nc.dram_tensor("bq", [128, 4], f32, kind="ExternalInput")
    bk_d = nc.dram_tensor("bk", [128, 4], f32, kind="ExternalInput")
    bv_d = nc.dram_tensor("bv", [128, 4], f32, kind="ExternalInput")
    bo_d = nc.dram_tensor("bo", [1, D], bf16, kind="ExternalInput")
    out = nc.dram_tensor("out", [S, D], f32, kind="ExternalOutput")

    wq_r = wq[:, :].rearrange("(ko ki) n -> ki ko n", ki=128)  # [128,8,512]
    wk_r = wk[:, :].rearrange("(ko ki) n -> ki ko n", ki=128)
    wv_r = wv[:, :].rearrange("(ko ki) n -> ki ko n", ki=128)
    wo_r = wo[:, :].rearrange("(ko ki) n -> ki ko n", ki=128)  # [128,4,1024]
    our = out[:, :].rearrange("(so si) d -> si so d", si=128)

    with tile.TileContext(nc) as tc, ExitStack() as ctx:
        pers = ctx.enter_context(tc.tile_pool(name="pers", bufs=1))
        qt = pers.tile([128, 4, S], bf16, name="qt")  # Q^T: dim x seq
        ktt = pers.tile([128, 4, S], bf16, name="ktt")  # K^T: dim x seq
        vaug = pers.tile([128, 16, 8, 65], bf16, name="vaug")  # V + ones col
        ones_row = pers.tile([1, 128], bf16, name="ones_row")
        bo_bc = pers.tile([128, D], f32, name="bo_bc")
        bo_row = pers.tile([1, D], bf16, name="bo_row")
        bq_sb = pers.tile([128, 4], f32, name="bq_sb")
        bk_sb = pers.tile([128, 4], f32, name="bk_sb")
        bv_sb = pers.tile([128, 4], f32, name="bv_sb")

        # ---- constants / small inputs ----
        nc.vector.memset(ones_row, 1.0)
        nc.gpsimd.memset(vaug[:, :, :, 64:65], 1.0)
        nc.sync.dma_start(bq_sb[:, :], bq_d[:, :])
        nc.sync.dma_start(bk_sb[:, :], bk_d[:, :])
        nc.sync.dma_start(bv_sb[:, :], bv_d[:, :])
        nc.sync.dma_start(bo_row[:, :], bo_d[:, :])

        # bo broadcast across partitions via ones-vector matmul
        with tc.tile_pool(name="initps", bufs=2, space="PSUM") as initps:
            for nb in range(2):
                pb = initps.tile([128, 512], f32, tag="initp")
                nc.tensor.matmul(
                    pb,
                    lhsT=ones_row[:, :],
                    rhs=bo_row[:, nb * 512 : (nb + 1) * 512],
                    start=True,
                    stop=True,
                )
                nc.any.tensor_copy(bo_bc[:, nb * 512 : (nb + 1) * 512], pb)

        # ---- fused pipeline ----
        # Query block qb's attention needs K/V/Q only for seq chunks <= qb
        # (causal), so QKV projection of chunk qb is emitted immediately
        # before attention on block qb.  This interleaves the PE-heavy
        # projection work with the ACT-heavy exp work of earlier blocks.
        late = ctx.enter_context(tc.tile_pool(name="late", bufs=1))
        ctxT = late.tile([128, 4, S], bf16, name="ctxT")
        wo_sb = late.tile([128, 4, D], bf16, name="wo_sb")
        wq_sb = late.tile([128, 8, DC], bf16, name="wq_sb")
        wk_sb = late.tile([128, 8, DC], bf16, name="wk_sb")
        wv_sb = late.tile([128, 8, DC], bf16, name="wv_sb")
        with (
            tc.tile_pool(name="xt", bufs=3) as xt_pool,
            tc.tile_pool(name="ptp", bufs=32) as pt_pool,
            tc.tile_pool(name="pps", bufs=2, space="PSUM") as pps,
            tc.tile_pool(name="sps", bufs=2, space="PSUM") as sps,
            tc.tile_pool(name="ups", bufs=2, space="PSUM") as ups,
            tc.tile_pool(name="smp", bufs=4) as smp,
            tc.tile_pool(name="osb", bufs=4) as osb_pool,
        ):
            def emit_xt(sb):
                """X^T DMA transposes for seq chunk sb."""
                ssl = slice(sb * 512, (sb + 1) * 512)
                xt_chunk = xt_pool.tile([128, 8, 512], bf16, tag="xt")
                for kd in range(8):
                    nc.sync.dma_start_transpose(
                        xt_chunk[:, kd, :], x[ssl, kd * 128 : (kd + 1) * 128]
                    )
                return xt_chunk

            def emit_qkv_chunk(sb, xt_chunk):
                """Q/K/V projections for seq chunk sb."""
                ssl = slice(sb * 512, (sb + 1) * 512)
                for m in range(4):  # output dim tiles (heads 2m, 2m+1)
                    pq = pps.tile([128, 512], f32, tag="pj")
                    for kd in range(8):
                        nc.tensor.matmul(
                            pq,
                            lhsT=wq_sb[:, kd, m * 128 : (m + 1) * 128],
                            rhs=xt_chunk[:, kd, :],
                            start=(kd == 0),
                            stop=(kd == 7),
                        )
                    nc.any.tensor_scalar_add(qt[:, m, ssl], pq, bq_sb[:, m : m + 1])
                    pk = pps.tile([128, 512], f32, tag="pj")
                    for kd in range(8):
                        nc.tensor.matmul(
                            pk,
                            lhsT=wk_sb[:, kd, m * 128 : (m + 1) * 128],
                            rhs=xt_chunk[:, kd, :],
                            start=(kd == 0),
                            stop=(kd == 7),
                        )
                    nc.any.tensor_scalar_add(ktt[:, m, ssl], pk, bk_sb[:, m : m + 1])
                for sv in range(4):  # V rows for this chunk (no bias here)
                    pv = pps.tile([128, 512], f32, tag="pj")
                    for kd in range(8):
                        nc.tensor.matmul(
                            pv,
                            lhsT=xt_chunk[:, kd, sv * 128 : (sv + 1) * 128],
                            rhs=wv_sb[:, kd, :],
                            start=(kd == 0),
                            stop=(kd == 7),
                        )
                    nc.any.tensor_copy(
                        vaug[:, sb * 4 + sv, :, 0:64],
                        pv[:, :].rearrange("p (h i) -> p h i", h=8),
                    )

            def emit_scores(hp, qb):
                """Score matmuls + exp for one (head-pair, query-block).

                Both heads of the pair go into one [128, 2, 512] PSUM tile
                (2 banks) so a single ACT exp covers them.  Diagonal
                k-tiles compute only their valid query columns; the
                in-tile triangle is zeroed with affine_select (valid iff
                p <= local f) directly on the bf16 P^T tile.
                """
                nkt = 4 * qb + 4
                tiles = []
                for kti in range(nkt):
                    oi = kti - 4 * qb
                    qoff = max(oi, 0) * 128
                    w = 512 - qoff
                    ps = sps.tile([128, 2, 512], f32, tag="s")
                    for h2 in range(2):
                        base = h2 * 64
                        nc.tensor.matmul(
                            ps[:, h2, :w],
                            lhsT=ktt[
                                base : base + 64, hp, kti * 128 : (kti + 1) * 128
                            ],
                            rhs=qt[
                                base : base + 64, hp,
                                qb * 512 + qoff : (qb + 1) * 512,
                            ],
                            start=True,
                            stop=True,
                        )
                    p_t = pt_pool.tile([128, 2, 512], bf16, tag="p")
                    nc.scalar.activation(p_t[:, :, :w], ps[:, :, :w], Exp, scale=0.125)
                    if oi >= 0:
                        nc.gpsimd.affine_select(
                            out=p_t[:, :, :w],
                            in_=p_t[:, :, :w],
                            compare_op=mybir.AluOpType.is_ge,
                            fill=0.0,
                            base=0,
                            channel_multiplier=-1,
                            pattern=[[0, 2], [1, w]],
                        )
                    tiles.append((kti, qoff, w, p_t))
                return tiles

            def emit_ctx(hp, qb, tiles):
                """P^T @ V accumulation + softmax normalization for a block."""
                nkt = len(tiles)
                qsl = slice(qb * 512, (qb + 1) * 512)
                for h2 in range(2):
                    base = h2 * 64
                    u = ups.tile([65, 512], f32, tag="u")
                    for kti, qoff, w, p_t in tiles:
                        nc.tensor.matmul(
                            u[:, qoff : qoff + w],
                            lhsT=vaug[:, kti, 2 * hp + h2, :],
                            rhs=p_t[:, h2, :w],
                            start=(kti == 0),
                            stop=(kti == nkt - 1),
                        )
                    rec = smp.tile([1, 512], bf16, tag="rec")
                    with nc.allow_low_precision(
                        reason="softmax 1/sum rounded to bf16; ~0.4% rel, "
                        "within tolerance"
                    ):
                        nc.vector.reciprocal(rec, u[64:65, :])
                    pb_sb = smp.tile([64, 512], bf16, tag="pbs")
                    nc.gpsimd.partition_broadcast(pb_sb[:, :], rec[:, :])
                    dst = ctxT[base : base + 64, hp, qsl]
                    nc.vector.tensor_mul(dst, u[0:64, :], pb_sb)
                    nc.any.tensor_scalar_add(
                        dst, dst, bv_sb[base : base + 64, hp : hp + 1]
                    )

            def emit_outproj(qb):
                """Output projection for the 4 seq tiles of query block qb."""
                for ms in range(qb * 4, qb * 4 + 4):
                    for nb in range(2):
                        po = pps.tile([128, 512], f32, tag="pj")
                        for kd in range(4):
                            nc.tensor.matmul(
                                po,
                                lhsT=ctxT[:, kd, ms * 128 : (ms + 1) * 128],
                                rhs=wo_sb[:, kd, nb * 512 : (nb + 1) * 512],
                                start=(kd == 0),
                                stop=(kd == 3),
                            )
                        ot = osb_pool.tile([128, 512], f32, tag="ot")
                        nc.vector.tensor_add(
                            ot, po, bo_bc[:, nb * 512 : (nb + 1) * 512]
                        )
                        nc.sync.dma_start(
                            our[:, ms, nb * 512 : (nb + 1) * 512], ot
                        )

            # Software pipeline: block N's scores are emitted before block
            # N-1's ctx matmuls so the PE has score work while ACT runs
            # the exps of the previous block.  QKV for chunk qb is emitted
            # right before the attention blocks that first need it, and
            # the output projection for a query block follows its last
            # head-pair.
            prev = None
            nc.sync.dma_start(wq_sb[:, :, :], wq_r)
            nc.sync.dma_start(wk_sb[:, :, :], wk_r)
            nc.sync.dma_start(wv_sb[:, :, :], wv_r)
            for qb in range(4):
                xt_chunk = emit_xt(qb)
                emit_qkv_chunk(qb, xt_chunk)
                if qb == 0:
                    nc.sync.dma_start(wo_sb[:, :, :], wo_r)
                for hp in range(4):
                    tiles = emit_scores(hp, qb)
                    if prev is not None:
                        emit_ctx(*prev)
                        if prev[0] == 3 and prev[1] != qb:
                            emit_outproj(prev[1])
                    prev = (hp, qb, tiles)
            emit_ctx(*prev)
            emit_outproj(3)

    nc.finalize()
    return nc


def _get_nc():
    global _nc_cache
    if _nc_cache is None:
        _nc_cache = _build_bass()
    return _nc_cache


def make_in_maps(inputs, Wq, bq, Wk, bk, Wv, bv, Wo, bo):
    import ml_dtypes

    bf = ml_dtypes.bfloat16
    inputs = np.asarray(inputs, dtype=np.float32)
    Wq, Wk, Wv, Wo = (np.asarray(a, dtype=np.float32) for a in (Wq, Wk, Wv, Wo))
    bq, bk, bv, bo = (np.asarray(a, dtype=np.float32) for a in (bq, bk, bv, bo))
    in_maps = []
    for c in range(N_CORES):
        b = c // 2
        lo = (c % 2) * DC
        hi = lo + DC
        in_maps.append(
            {
                "x": np.ascontiguousarray(inputs[b]).astype(bf),
                "wq": np.ascontiguousarray(Wq[:, lo:hi]).astype(bf),
                "wk": np.ascontiguousarray(Wk[:, lo:hi]).astype(bf),
                "wv": np.ascontiguousarray(Wv[:, lo:hi]).astype(bf),
                "wo": np.ascontiguousarray(Wo[lo:hi, :]).astype(bf),
                "bq": np.ascontiguousarray(bq[lo:hi].reshape(4, 128).T),
                "bk": np.ascontiguousarray(bk[lo:hi].reshape(4, 128).T),
                "bv": np.ascontiguousarray(bv[lo:hi].reshape(4, 128).T),
                "bo": (
                    bo.reshape(1, D).astype(bf)
                    if c % 2 == 0
                    else np.zeros((1, D), dtype=bf)
                ),
            }
        )
    return in_maps


def run(in_maps, trace=False):
    from concourse.bass_utils import run_bass_kernel_spmd

    nc = _get_nc()
    res = run_bass_kernel_spmd(
        nc, in_maps, core_ids=list(range(N_CORES)), trace=trace
    )
    parts = [r["out"] for r in res.results]
    full = np.stack(
        [parts[2 * b] + parts[2 * b + 1] for b in range(B)]
    ).astype(np.float32)
    return full, res


def kernel(inputs, Wq, bq, Wk, bk, Wv, bv, Wo, bo):
    in_maps = make_in_maps(inputs, Wq, bq, Wk, bk, Wv, bv, Wo, bo)
    full, _ = run(in_maps, trace=False)
    return full



# revision 10
# speedup vs baseline: 1.2579x; 1.2579x over previous
"""Multi-head causal attention (B=4, S=2048, D=1024, H=16) on 8 TRN2 NeuronCores.

Sharding: 2 cores per batch element, 8 heads (512 dims) per core.
Each core computes QKV projections for its head slice, causal attention,
and a partial output projection (its 512 rows of Wo). The host sums the
two partial outputs per batch element and adds (bo + bv @ Wo) once
(softmax rows sum to 1, so the bv term passes through Wo exactly).

Compute dtype: bf16 matmul inputs with fp32 PSUM accumulation.

Per-core dataflow (layouts chosen so nothing needs an on-device
transpose; X^T is built host-side):
  1. X^T [d x seq] chunks DMA'd straight in (bf16).
  2. Q^T, K^T [dim(512) x seq] = W^T @ X^T, V [seq x dim] stored bf16
     in `vone` [128, kti, 9, 64]: blocks 0-7 are the 8 heads' V dims,
     block 8 is all-ones.  The ctx matmul's stationary operand for head
     h is the strided slice (blocks h and 8), so the P^T @ V_aug matmul
     emits ctx^T on partitions 0-63 and the softmax row sums replicated
     on partitions 64-127 (broadcast for free).
  3. Per head-pair/query-block: S^T tiles [ks,qs] = K @ Q^T (diagonal
     k-tiles first), exp on ACT -> bf16 P^T, causal triangle zeroed by
     gpsimd affine_select on the 4 diagonal tiles.  ctx^T accumulated
     as V_aug^T @ P^T.  1/sums via the fast DVE reciprocal on the
     replicated-sums partitions, one tensor_mul normalizes.
  4. out_partial = ctx^T.T @ Wo, copy to SBUF bf16, DMA out (biases
     are applied host-side in fp32).
Schedule: block N's scores are emitted before block N-1's ctx matmuls
(PE has score work while ACT runs block N-1's exps); QKV projection of
seq chunk qb+1 is spread across the 4 head-pair iterations of query
block qb; the output projection of block qb trails by two head-pair
iterations so its ctxT inputs' normalization is off the critical path.
"""

import sys

import numpy as np


def _ensure_concourse():
    try:
        import concourse  # noqa: F401
    except ImportError:
        sys.path.insert(0, "/opt/trn_rl_repo")


_ensure_concourse()

B, S, D, H, HD = 4, 2048, 1024, 16, 64
DC = 512  # dims (= 8 heads) per core
N_CORES = 8

_nc_cache = None


def _build_bass():
    from contextlib import ExitStack

    import concourse.mybir as mybir
    import concourse.tile as tile
    from concourse import bacc

    f32 = mybir.dt.float32
    bf16 = mybir.dt.bfloat16
    Exp = mybir.ActivationFunctionType.Exp

    nc = bacc.Bacc(None, target_bir_lowering=False)

    xt_d = nc.dram_tensor("xt", [D, S], bf16, kind="ExternalInput")
    wq = nc.dram_tensor("wq", [D, DC], bf16, kind="ExternalInput")
    wk = nc.dram_tensor("wk", [D, DC], bf16, kind="ExternalInput")
    wv = nc.dram_tensor("wv", [D, DC], bf16, kind="ExternalInput")
    wo = nc.dram_tensor("wo", [DC, D], bf16, kind="ExternalInput")
    bq_d = nc.dram_tensor("bq", [128, 4], f32, kind="ExternalInput")
    bk_d = nc.dram_tensor("bk", [128, 4], f32, kind="ExternalInput")
    out = nc.dram_tensor("out", [S, D], bf16, kind="ExternalOutput")

    xt_r = xt_d[:, :].rearrange("(ko ki) s -> ki ko s", ki=128)  # [128,8,S]
    wq_r = wq[:, :].rearrange("(ko ki) n -> ki ko n", ki=128)  # [128,8,512]
    wk_r = wk[:, :].rearrange("(ko ki) n -> ki ko n", ki=128)
    wv_r = wv[:, :].rearrange("(ko ki) n -> ki ko n", ki=128)
    wo_r = wo[:, :].rearrange("(ko ki) n -> ki ko n", ki=128)  # [128,4,1024]
    our = out[:, :].rearrange("(so si) d -> si so d", si=128)

    with tile.TileContext(nc) as tc, ExitStack() as ctx:
        pers = ctx.enter_context(tc.tile_pool(name="pers", bufs=1))
        qt = pers.tile([128, 4, S], bf16, name="qt")  # Q^T: dim x seq
        ktt = pers.tile([128, 4, S], bf16, name="ktt")  # K^T: dim x seq
        # Per head: 64 ones-columns then the 64 V dims.  The ctx matmul's
        # PSUM output then carries the softmax row sums replicated on
        # partitions 0-63 (a ready-made broadcast for the normalization
        # multiply, base-partition 0 as the fast-reciprocal custom DVE op
        # requires) and ctx^T on partitions 64-127.
        vaug = pers.tile([128, 16, 8, 128], bf16, name="vaug")
        ctxT = pers.tile([128, 4, S], bf16, name="ctxT")
        wq_sb = pers.tile([128, 8, DC], bf16, name="wq_sb")
        wk_sb = pers.tile([128, 8, DC], bf16, name="wk_sb")
        wv_sb = pers.tile([128, 8, DC], bf16, name="wv_sb")
        wo_sb = pers.tile([128, 4, D], bf16, name="wo_sb")
        bq_sb = pers.tile([128, 4], f32, name="bq_sb")
        bk_sb = pers.tile([128, 4], f32, name="bk_sb")

        nc.vector.memset(vaug[:, :, :, 0:64], 1.0)
        nc.sync.dma_start(bq_sb[:, :], bq_d[:, :])
        nc.sync.dma_start(bk_sb[:, :], bk_d[:, :])

        with (
            tc.tile_pool(name="xt", bufs=2) as xt_pool,
            tc.tile_pool(name="ptp", bufs=32) as pt_pool,
            tc.tile_pool(name="pps", bufs=2, space="PSUM") as pps,
            tc.tile_pool(name="sps", bufs=2, space="PSUM") as sps,
            tc.tile_pool(name="ups", bufs=2, space="PSUM") as ups,
            tc.tile_pool(name="recp", bufs=4) as rec_pool,
            tc.tile_pool(name="osb", bufs=4) as osb_pool,
        ):
            xt_chunks = {}

            def emit_xt_dma(sb):
                xt_chunk = xt_pool.tile([128, 8, 512], bf16, tag="xt")
                nc.sync.dma_start(xt_chunk, xt_r[:, :, sb * 512 : (sb + 1) * 512])
                xt_chunks[sb] = xt_chunk

            def emit_qkv_group(sb, g):
                """One of 12 projection groups for seq chunk sb.

                Groups 0-3: Q^T m-tile g; 4-7: K^T m-tile g-4;
                8-11: V rows tile g-8.
                """
                ssl = slice(sb * 512, (sb + 1) * 512)
                xt_chunk = xt_chunks[sb]
                if g < 8:
                    m = g % 4
                    w_sb, dst, b_sb = (
                        (wq_sb, qt, bq_sb) if g < 4 else (wk_sb, ktt, bk_sb)
                    )
                    p = pps.tile([128, 512], f32, tag="pj")
                    for kd in range(8):
                        nc.tensor.matmul(
                            p,
                            lhsT=w_sb[:, kd, m * 128 : (m + 1) * 128],
                            rhs=xt_chunk[:, kd, :],
                            start=(kd == 0),
                            stop=(kd == 7),
                        )
                    nc.vector.tensor_scalar_add(dst[:, m, ssl], p, b_sb[:, m : m + 1])
                else:
                    sv = g - 8
                    p = pps.tile([128, 512], f32, tag="pj")
                    for kd in range(8):
                        nc.tensor.matmul(
                            p,
                            lhsT=xt_chunk[:, kd, sv * 128 : (sv + 1) * 128],
                            rhs=wv_sb[:, kd, :],
                            start=(kd == 0),
                            stop=(kd == 7),
                        )
                    nc.vector.tensor_copy(
                        vaug[:, sb * 4 + sv, :, 64:128],
                        p[:, :].rearrange("p (h i) -> p h i", h=8),
                    )

            def emit_scores(hp, qb):
                """Score matmuls + exp for one (head-pair, query-block).

                Both heads of the pair go into one [128, 2, 512] PSUM tile
                (2 banks) so a single ACT exp covers them; the two matmuls
                target disjoint PE row groups (partitions 0-63 / 64-127)
                and can overlap.  Diagonal k-tiles run FIRST so their
                gpsimd triangle-mask is done well before the ctx matmuls
                need the tiles; the in-tile triangle is zeroed with
                affine_select (valid iff p <= local f) on the bf16 tile.
                """
                tiles = []
                ktis = list(range(4 * qb, 4 * qb + 4)) + list(range(4 * qb))
                for kti in ktis:
                    oi = kti - 4 * qb
                    qoff = max(oi, 0) * 128
                    w = 512 - qoff
                    ps = sps.tile([128, 2, 512], f32, tag="s")
                    for h2 in range(2):
                        base = h2 * 64
                        nc.tensor.matmul(
                            ps[:, h2, :w],
                            lhsT=ktt[
                                base : base + 64, hp, kti * 128 : (kti + 1) * 128
                            ],
                            rhs=qt[
                                base : base + 64, hp,
                                qb * 512 + qoff : (qb + 1) * 512,
                            ],
                            start=True,
                            stop=True,
                        )
                    p_t = pt_pool.tile([128, 2, 512], bf16, tag="p")
                    nc.scalar.activation(p_t[:, :, :w], ps[:, :, :w], Exp, scale=0.125)
                    if oi >= 0:
                        nc.gpsimd.affine_select(
                            out=p_t[:, :, :w],
                            in_=p_t[:, :, :w],
                            compare_op=mybir.AluOpType.is_ge,
                            fill=0.0,
                            base=0,
                            channel_multiplier=-1,
                            pattern=[[0, 2], [1, w]],
                        )
                    tiles.append((kti, qoff, w, p_t))
                return tiles

            def emit_ctx(hp, qb, tiles):
                """P^T @ V_aug accumulation + softmax normalization.

                u rows 0-63 hold the softmax row sums replicated (ones
                block of vaug); rows 64-127 are ctx^T for the head.  The
                fast DVE reciprocal on rows 0-63 directly yields the
                broadcast 1/sums; one tensor_mul normalizes.
                """
                nkt = len(tiles)
                qsl = slice(qb * 512, (qb + 1) * 512)
                for h2 in range(2):
                    h = 2 * hp + h2
                    u = ups.tile([128, 512], f32, tag="u")
                    for j, (kti, qoff, w, p_t) in enumerate(tiles):
                        nc.tensor.matmul(
                            u[:, qoff : qoff + w],
                            lhsT=vaug[:, kti, h, :],
                            rhs=p_t[:, h2, :w],
                            start=(j == 0),
                            stop=(j == nkt - 1),
                        )
                    rec = rec_pool.tile([64, 512], f32, tag="rec")
                    nc.vector.reciprocal_approx_fast(rec, u[0:64, :])
                    nc.vector.tensor_mul(
                        ctxT[h2 * 64 : h2 * 64 + 64, hp, qsl], u[64:128, :], rec
                    )

            def emit_outproj(qb):
                """Output projection for the 4 seq tiles of query block qb."""
                for ms in range(qb * 4, qb * 4 + 4):
                    for nb in range(2):
                        po = pps.tile([128, 512], f32, tag="pj")
                        for kd in range(4):
                            nc.tensor.matmul(
                                po,
                                lhsT=ctxT[:, kd, ms * 128 : (ms + 1) * 128],
                                rhs=wo_sb[:, kd, nb * 512 : (nb + 1) * 512],
                                start=(kd == 0),
                                stop=(kd == 3),
                            )
                        ot = osb_pool.tile([128, 512], bf16, tag="ot")
                        nc.vector.tensor_copy(ot, po)
                        nc.sync.dma_start(
                            our[:, ms, nb * 512 : (nb + 1) * 512], ot
                        )

            # ---- prologue: DMAs + chunk-0 projections ----
            emit_xt_dma(0)
            nc.sync.dma_start(wq_sb[:, :, :], wq_r)
            nc.sync.dma_start(wk_sb[:, :, :], wk_r)
            nc.sync.dma_start(wv_sb[:, :, :], wv_r)
            nc.sync.dma_start(wo_sb[:, :, :], wo_r)
            for g in range(12):
                emit_qkv_group(0, g)

            # ---- pipelined attention ----
            prev = prev2 = None
            for qb in range(4):
                if qb < 3:
                    emit_xt_dma(qb + 1)
                for hp in range(4):
                    tiles = emit_scores(hp, qb)
                    if qb < 3:
                        for g in range(3 * hp, 3 * hp + 3):
                            emit_qkv_group(qb + 1, g)
                    if prev is not None:
                        emit_ctx(*prev)
                    if prev2 is not None and prev2[0] == 3:
                        emit_outproj(prev2[1])
                    prev2 = prev
                    prev = (hp, qb, tiles)
            emit_ctx(*prev)
            emit_outproj(3)

    nc.finalize()
    return nc


def _get_nc():
    global _nc_cache
    if _nc_cache is None:
        _nc_cache = _build_bass()
    return _nc_cache


def make_in_maps(inputs, Wq, bq, Wk, bk, Wv, bv, Wo, bo):
    import ml_dtypes

    bf = ml_dtypes.bfloat16
    inputs = np.asarray(inputs, dtype=np.float32)
    Wq, Wk, Wv, Wo = (np.asarray(a, dtype=np.float32) for a in (Wq, Wk, Wv, Wo))
    bq, bk = (np.asarray(a, dtype=np.float32) for a in (bq, bk))
    in_maps = []
    for c in range(N_CORES):
        b = c // 2
        lo = (c % 2) * DC
        hi = lo + DC
        in_maps.append(
            {
                "xt": np.ascontiguousarray(inputs[b].T).astype(bf),
                "wq": np.ascontiguousarray(Wq[:, lo:hi]).astype(bf),
                "wk": np.ascontiguousarray(Wk[:, lo:hi]).astype(bf),
                "wv": np.ascontiguousarray(Wv[:, lo:hi]).astype(bf),
                "wo": np.ascontiguousarray(Wo[lo:hi, :]).astype(bf),
                "bq": np.ascontiguousarray(bq[lo:hi].reshape(4, 128).T),
                "bk": np.ascontiguousarray(bk[lo:hi].reshape(4, 128).T),
            }
        )
    return in_maps


def run(in_maps, bias_full, trace=False):
    from concourse.bass_utils import run_bass_kernel_spmd

    nc = _get_nc()
    res = run_bass_kernel_spmd(
        nc, in_maps, core_ids=list(range(N_CORES)), trace=trace
    )
    parts = [np.asarray(r["out"], dtype=np.float32) for r in res.results]
    full = np.stack([parts[2 * b] + parts[2 * b + 1] for b in range(B)])
    full += bias_full[None, None, :]
    return full, res


def _bias_full(Wo, bv, bo):
    # softmax rows sum to 1: ctx = attn @ (V + bv) = attn @ V + bv, and the
    # bv term passes through the output projection exactly.
    Wo = np.asarray(Wo, dtype=np.float32)
    bv = np.asarray(bv, dtype=np.float32)
    bo = np.asarray(bo, dtype=np.float32)
    return bo + bv @ Wo


def kernel(inputs, Wq, bq, Wk, bk, Wv, bv, Wo, bo):
    in_maps = make_in_maps(inputs, Wq, bq, Wk, bk, Wv, bv, Wo, bo)
    full, _ = run(in_maps, _bias_full(Wo, bv, bo), trace=False)
    return full
